# revision 2
# baseline (speedup 1.0000x reference)
"""Trainium2 Bass kernel for NeuralComplexityLoss (sample-entropy MSE).

Contract: kernel(predictions, targets) -> np.float32 scalar (shape ()),
matching reference.reference(). Self-contained: hardcodes shapes/sharding.

Strategy (diagonal layout)
--------------------------
128 signals (64 pred + 64 targ, each length T=1024, host-normalized) are
sharded 16 per core across 8 NeuronCores. For a signal x, the sample-entropy
match counts over the strict upper triangle are accumulated per DIAGONAL
d = j - i:

  B[d, i] = [ |x_i - x_{i+d}| <= R ]          (one threshold per element)
  cnt2   += sum_i B[d, i] * B[d, i+1]         (M=2 template match)
  cnt3   += sum_i B[d, i] * B[d, i+1] * B[d, i+2]

Per 32-diagonal block, 4 signals are packed into the 128 SBUF partitions
(lane = dd*4 + sig). Tiles Z[l, n] = xpad[sig, n + dd] (DMA shift-gather)
and X0[l, i] = xpad[sig, i] (DMA broadcast) give, per block bb:

  S  = X0 - Z[:, 1+32bb : ...]     (fp32 subtract, GPSIMD or DVE)
  A  = |S|                         (ACT Abs, fp32 -> fp16)
  B  = (A <= R)                    (DVE tensor_scalar, fp16 2x mode)
  c2 = B0 * B1  (+row-sum accum)   (DVE stt, fp16)
  c3 = c2 * B2  (+row-sum accum)   (DVE stt, fp16)

x is padded with 60000.0 beyond T so out-of-range template positions give
B = 0 automatically; the single per-diagonal spurious c2 term (templates
ending exactly at the boundary) is subtracted on the host in O(N).
Row sums land in a [128, 256] stats tile; a single PE matmul with a
(k%4==m) selector reduces partitions per-signal -> [4, 256] output.
Host forms matches_m = 2*(cnt2 - spur2) + N, matches_m1 = 2*cnt3 + N,
then entropies and the final MSE.
"""

import os

import numpy as np

B, C, T = 4, 16, 1024
M = 2
R = 0.2
EPS = 1e-8
N = T - M                      # 1022 templates
NCORES = 8
NSIG = 2 * B * C               # 128 signals total
S_PER_CORE = NSIG // NCORES    # 16
PADLEN = 1152                  # padded signal length (>= 1024 + 127)
PADV = 60000.0                 # pad value (huge -> no match, fp16-safe)
NB = 32                        # diagonal blocks per group (32 diagonals each)
NG = 4                         # signal groups of 4 per core

_CACHE = {}
LAST_RESULTS = None


def _split_excess_waits(nc, maxw=1):
    """This walrus codegen accepts only one sync-wait per instruction:
    hoist extras onto preceding single-wait NOPs on the same engine."""
    import bass_rust
    import concourse.mybir as mybir

    n_split = 0
    for bb in nc.main_func.blocks:
        insts = bb.instructions
        i = 0
        while i < len(insts):
            ins = insts[i]
            si = ins.sync_info
            waits = list(si.on_wait) if si is not None and si.on_wait else []
            if len(waits) > maxw:
                extra, keep = waits[:-maxw], waits[-maxw:]
                nops = []
                for j, w in enumerate(extra):
                    nop = bass_rust.InstNoOp(
                        name=f"{ins.name}-wsplit{j}", ins=[], outs=[]
                    )
                    nop.engine = ins.engine
                    nop.sync_info = mybir.SyncInfo(on_wait=[w], on_update=[])
                    nops.append(nop)
                si.on_wait = keep
                insts[i:i] = nops
                i += len(nops)
                n_split += 1
            i += 1
    return n_split


def _build():
    import concourse.bass as bass
    import concourse.tile as tile
    from concourse import mybir
    from concourse.alu_op_type import AluOpType

    f32 = mybir.dt.float32
    f16 = mybir.dt.float16
    seng = os.environ.get("KSENG", "pool")  # engine for the fp32 subtract

    nc = bass.Bass(trn_type="TRN2", num_devices=NCORES)
    x = nc.dram_tensor("x", [S_PER_CORE, PADLEN], f32, kind="ExternalInput")
    out = nc.dram_tensor("cnt", [4, 2 * NB * NG], f32, kind="ExternalOutput")

    sel_np = np.zeros((128, 4), dtype=np.float32)
    sel_np[np.arange(128), np.arange(128) % 4] = 1.0
    sel_dram = nc.inline_tensor(sel_np, name="sel")

    xa = x.ap()
    with tile.TileContext(nc) as tc:
        with (
            tc.tile_pool(name="singles", bufs=1) as singles,
            tc.tile_pool(name="grp", bufs=2) as grp,
            tc.tile_pool(name="wrk", bufs=3) as wrk,
            tc.tile_pool(name="psum", bufs=1, space="PSUM") as psum,
        ):
            stats = singles.tile([128, 2 * NB * NG], f32)
            nc.vector.memset(stats, 0.0)
            selt = singles.tile([128, 4], f32)
            nc.sync.dma_start(out=selt, in_=sel_dram[:, :])

            for g in range(NG):
                goff = g * 4 * PADLEN
                # Z[l, n] = xpad[sig, n + dd], l = dd*4 + sig
                Z = grp.tile([128, 1024], f32)
                nc.sync.dma_start(
                    out=Z,
                    in_=bass.AP(
                        tensor=xa.tensor,
                        offset=xa.offset + goff,
                        ap=[[1, 32], [PADLEN, 4], [1, 1024]],
                    ),
                )
                # X0[l, i] = xpad[sig, i]
                X0 = grp.tile([128, 1023], f32)
                nc.sync.dma_start(
                    out=X0,
                    in_=bass.AP(
                        tensor=xa.tensor,
                        offset=xa.offset + goff,
                        ap=[[0, 32], [PADLEN, 4], [1, 1023]],
                    ),
                )
                for bb in range(NB):
                    WB = 1023 - 32 * bb
                    d0 = 1 + 32 * bb
                    S = wrk.tile([128, 1024], f32)
                    if seng == "pool":
                        nc.gpsimd.tensor_tensor(
                            out=S[:, 0:WB],
                            in0=X0[:, 0:WB],
                            in1=Z[:, d0 : d0 + WB],
                            op=AluOpType.subtract,
                        )
                    else:
                        nc.vector.tensor_tensor(
                            out=S[:, 0:WB],
                            in0=X0[:, 0:WB],
                            in1=Z[:, d0 : d0 + WB],
                            op=AluOpType.subtract,
                        )
                    A = wrk.tile([128, 1024], f16)
                    nc.scalar.activation(
                        out=A[:, 0:WB],
                        in_=S[:, 0:WB],
                        func=mybir.ActivationFunctionType.Abs,
                    )
                    Bt = wrk.tile([128, 1024], f16)
                    nc.vector.tensor_scalar(
                        out=Bt[:, 0:WB],
                        in0=A[:, 0:WB],
                        scalar1=float(R),
                        scalar2=None,
                        op0=AluOpType.is_le,
                    )
                    col2 = g * NB + bb
                    col3 = NB * NG + col2
                    c2t = wrk.tile([128, 1024], f16)
                    nc.vector.scalar_tensor_tensor(
                        out=c2t[:, 0 : WB - 1],
                        in0=Bt[:, 0 : WB - 1],
                        scalar=1.0,
                        in1=Bt[:, 1:WB],
                        op0=AluOpType.mult,
                        op1=AluOpType.mult,
                        accum_out=stats[:, col2 : col2 + 1],
                    )
                    c3t = wrk.tile([128, 1024], f16)
                    nc.vector.scalar_tensor_tensor(
                        out=c3t[:, 0 : WB - 2],
                        in0=c2t[:, 0 : WB - 2],
                        scalar=1.0,
                        in1=Bt[:, 2:WB],
                        op0=AluOpType.mult,
                        op1=AluOpType.mult,
                        accum_out=stats[:, col3 : col3 + 1],
                    )

            pt = psum.tile([4, 2 * NB * NG], f32)
            nc.tensor.matmul(pt, selt, stats, start=True, stop=True)
            red = singles.tile([4, 2 * NB * NG], f32)
            nc.scalar.copy(out=red, in_=pt)
            nc.sync.dma_start(out=out[:, :], in_=red)

    _split_excess_waits(nc)
    return nc


def _get_nc():
    if "nc" not in _CACHE:
        _CACHE["nc"] = _build()
    return _CACHE["nc"]


def _get_runner():
    """Cached jitted 8-core executor: xpad [128, PADLEN] f32 -> [NCORES, 4, 256]."""
    if "fn" in _CACHE:
        return _CACHE["fn"]
    import jax
    import numpy as _np
    from jax.sharding import Mesh, PartitionSpec
    from jax.experimental.shard_map import shard_map
    import concourse.mybir as mybir
    from concourse.bass2jax import (
        _bass_exec_p,
        install_neuronx_cc_hook,
        partition_id_tensor,
    )

    nc = _get_nc()
    install_neuronx_cc_hook()

    in_names, out_names, out_avals, zero_outs = [], [], [], []
    partition_name = nc.partition_id_tensor.name if nc.partition_id_tensor else None
    for alloc in nc.m.functions[0].allocations:
        if not isinstance(alloc, mybir.MemoryLocationSet):
            continue
        name = alloc.memorylocations[0].name
        if alloc.kind == "ExternalInput":
            if name != partition_name:
                in_names.append(name)
        elif alloc.kind == "ExternalOutput":
            shape = tuple(alloc.tensor_shape)
            dtype = mybir.dt.np(alloc.dtype)
            out_names.append(name)
            out_avals.append(jax.core.ShapedArray(shape, dtype))
            zero_outs.append(_np.zeros(shape, dtype))
    n_params = len(in_names)
    n_outs = len(out_avals)
    all_in_names = list(in_names) + list(out_names) + (
        [partition_name] if partition_name else []
    )

    def _body(*args):
        operands = list(args)
        if partition_name is not None:
            operands.append(partition_id_tensor())
        return tuple(
            _bass_exec_p.bind(
                *operands,
                out_avals=tuple(out_avals),
                in_names=tuple(all_in_names),
                out_names=tuple(out_names),
                lowering_input_output_aliases=(),
                sim_require_finite=True,
                sim_require_nnan=True,
                nc=nc,
            )
        )

    devices = jax.devices("axon")[:NCORES]
    mesh = Mesh(np.asarray(devices), ("core",))
    in_specs = (PartitionSpec("core"),) * (n_params + n_outs)
    out_specs = (PartitionSpec("core"),) * n_outs
    fn = jax.jit(
        shard_map(
            _body, mesh=mesh, in_specs=in_specs, out_specs=out_specs, check_rep=False
        ),
        keep_unused=True,
    )
    concat_zeros = [
        np.zeros((NCORES * z.shape[0], *z.shape[1:]), z.dtype) for z in zero_outs
    ]

    def run(xpad):
        out = fn(xpad, *concat_zeros)
        arr = np.asarray(out[0])  # [NCORES*4, 256]
        return arr.reshape(NCORES, 4, 2 * NB * NG)

    _CACHE["fn"] = run
    return run


def kernel(predictions, targets, _trace=False):
    global LAST_RESULTS

    preds = np.asarray(predictions, dtype=np.float32).reshape(B * C, T)
    targs = np.asarray(targets, dtype=np.float32).reshape(B * C, T)
    xall = np.concatenate([preds, targs], axis=0)  # [128, T]

    mu = xall.mean(axis=1, dtype=np.float64)
    sd = xall.std(axis=1, ddof=1, dtype=np.float64)
    xhat = ((xall - mu[:, None]) / (sd[:, None] + EPS)).astype(np.float32)

    xpad = np.full((NSIG, PADLEN), PADV, dtype=np.float32)
    xpad[:, :T] = xhat

    run = _get_runner()
    res = run(np.ascontiguousarray(xpad))
    LAST_RESULTS = res

    # Host-side spurious-c2 correction (device B is fp16-rounded; mirror it)
    S1 = (xhat[:, 0:N] - xhat[:, N : N + 1]).astype(np.float16)
    S2 = (xhat[:, 1 : N + 1] - xhat[:, N + 1 : N + 2]).astype(np.float16)
    spur2 = (
        (np.abs(S1).astype(np.float32) <= np.float32(R))
        & (np.abs(S2).astype(np.float32) <= np.float32(R))
    ).sum(axis=1)  # [128]

    ents = np.zeros(NSIG, dtype=np.float64)
    for c in range(NCORES):
        o = res[c].astype(np.float64)  # [4, 256]
        for sl in range(S_PER_CORE):
            g, sig = sl // 4, sl % 4
            cnt2 = o[sig, g * NB : (g + 1) * NB].sum()
            cnt3 = o[sig, NB * NG + g * NB : NB * NG + (g + 1) * NB].sum()
            sg = S_PER_CORE * c + sl
            m = 2.0 * (cnt2 - spur2[sg]) + N
            m1 = 2.0 * cnt3 + N
            ratio = m1 / max(m, 1.0)
            ent = -np.log(max(ratio, 1e-30)) if (m > 0 and m1 > 0) else 0.0
            ents[sg] = ent

    ep = ents[: B * C].reshape(B, C)
    et = ents[B * C :].reshape(B, C)
    return np.array(np.mean((ep - et) ** 2), dtype=np.float32)


# revision 4
# speedup vs baseline: 1.1561x; 1.1561x over previous
"""Trainium2 Bass kernel for NeuralComplexityLoss (sample-entropy MSE).

Contract: kernel(predictions, targets) -> np.float32 scalar (shape ()),
matching reference.reference(). Self-contained: hardcodes shapes/sharding.

Strategy (diagonal layout, quad-batched, engine-balanced)
---------------------------------------------------------
128 signals (64 pred + 64 targ, length T=1024, host-normalized) are sharded
16 per core across 8 NeuronCores. Per signal, sample-entropy match counts
accumulate per diagonal d = j - i:

  B[d, i] = [ |x_i - x_{i+d}| <= R ]
  cnt2 += sum_i B[d,i] B[d,i+1],   cnt3 += sum_i B[d,i] B[d,i+1] B[d,i+2]

Packing: 4 signals x 32 diagonals per 128 SBUF partitions (lane = dd*4+sig).
The 32 diagonal-blocks pair into 8 constant-width quads {q,15-q,16+q,31-q}
(width 2116 incl. 2 pad columns per segment). x is padded with an increasing
ramp (50000+100k) so every out-of-range position yields |S| >> R.

Per (group, quad):
  S    = X0 - Z          fp32 -> fp16   (widest blocks on DVE, rest GPSIMD)
  A    = |S|             ACT Abs -> fp16
  then one of two count flavors (split tuned to balance engines):
   DVE-form:  B = (A<=R) [DVE 4x];  c2 = (A0<=R)*B1 [stt+accum];
              c3 = (A2<=R)*c2 [stt+accum]
   ACT-form:  m2 = max(A0,A1), m3 = max(m2,A2) [DVE tt 2x];
              sign-counts: accum sum(Sign(R-m)) on ACT; host decodes
              cnt = (acc + 32*W)/2  (no exact ties: 0.2 not fp16-exact)

Row sums land in stats [128, 64]; one PE matmul with a (k%4==m) selector
gives per-signal sums [4, 64]. Host subtracts the single per-diagonal
spurious c2 term (O(N), exact fp16 mirror), forms matches_m = 2*(cnt2-spur)+N,
matches_m1 = 2*cnt3+N, entropies, and the final MSE.
"""

import os

import numpy as np

B, C, T = 4, 16, 1024
M = 2
R = 0.2
EPS = 1e-8
N = T - M                      # 1022 templates
NCORES = 8
NSIG = 2 * B * C               # 128 signals
S_PER_CORE = NSIG // NCORES    # 16
PADLEN = 1152
NG = 4                         # signal groups of 4 per core
NQ = 8                         # quads per group
QUADS = [[q, 15 - q, 16 + q, 31 - q] for q in range(NQ)]
QW = sum((1023 - 32 * b) + 2 for b in QUADS[0])  # 2116, same for all quads
N_DVE_FORM = int(os.environ.get("KPHI", "5"))    # quads 0..KPHI-1 use DVE-form
N_S_DVE = int(os.environ.get("KSDVE", "2"))      # widest blocks' S on DVE

_CACHE = {}
LAST_RESULTS = None


def _split_excess_waits(nc, maxw=1):
    """This walrus codegen accepts only one sync-wait per instruction:
    hoist extras onto preceding single-wait NOPs on the same engine."""
    import bass_rust
    import concourse.mybir as mybir

    n_split = 0
    for bb in nc.main_func.blocks:
        insts = bb.instructions
        i = 0
        while i < len(insts):
            ins = insts[i]
            si = ins.sync_info
            waits = list(si.on_wait) if si is not None and si.on_wait else []
            if len(waits) > maxw:
                extra, keep = waits[:-maxw], waits[-maxw:]
                nops = []
                for j, w in enumerate(extra):
                    nop = bass_rust.InstNoOp(
                        name=f"{ins.name}-wsplit{j}", ins=[], outs=[]
                    )
                    nop.engine = ins.engine
                    nop.sync_info = mybir.SyncInfo(on_wait=[w], on_update=[])
                    nops.append(nop)
                si.on_wait = keep
                insts[i:i] = nops
                i += len(nops)
                n_split += 1
            i += 1
    return n_split


def _build():
    import concourse.bass as bass
    import concourse.tile as tile
    from concourse import mybir
    from concourse.alu_op_type import AluOpType

    f32 = mybir.dt.float32
    f16 = mybir.dt.float16

    nc = bass.Bass(trn_type="TRN2", num_devices=NCORES)
    x = nc.dram_tensor("x", [S_PER_CORE, PADLEN], f32, kind="ExternalInput")
    out = nc.dram_tensor("cnt", [4, 2 * NQ * NG], f32, kind="ExternalOutput")

    sel_np = np.zeros((128, 4), dtype=np.float32)
    sel_np[np.arange(128), np.arange(128) % 4] = 1.0
    sel_dram = nc.inline_tensor(sel_np, name="sel")

    xa = x.ap()
    with tile.TileContext(nc) as tc:
        with (
            tc.tile_pool(name="singles", bufs=1) as singles,
            tc.tile_pool(name="grp", bufs=2) as grp,
            tc.tile_pool(name="wrk", bufs=3) as wrk,
            tc.tile_pool(name="psum", bufs=1, space="PSUM") as psum,
        ):
            stats = singles.tile([128, 2 * NQ * NG], f32)
            nc.vector.memset(stats, 0.0)
            rbias = singles.tile([128, 1], f32)
            nc.vector.memset(rbias, float(R))
            selt = singles.tile([128, 4], f32)
            nc.sync.dma_start(out=selt, in_=sel_dram[:, :])

            for g in range(NG):
                goff = g * 4 * PADLEN
                # Z[l, n] = xpad[sig, n + dd], l = dd*4 + sig
                Z = grp.tile([128, 1026], f32)
                nc.sync.dma_start(
                    out=Z,
                    in_=bass.AP(
                        tensor=xa.tensor,
                        offset=xa.offset + goff,
                        ap=[[1, 32], [PADLEN, 4], [1, 1026]],
                    ),
                )
                # X0[l, i] = xpad[sig, i]
                X0 = grp.tile([128, 1025], f32)
                nc.sync.dma_start(
                    out=X0,
                    in_=bass.AP(
                        tensor=xa.tensor,
                        offset=xa.offset + goff,
                        ap=[[0, 32], [PADLEN, 4], [1, 1025]],
                    ),
                )
                for q in range(NQ):
                    S = wrk.tile([128, QW], f16)
                    off = 0
                    for b in QUADS[q]:
                        WB = 1023 - 32 * b
                        d0 = 1 + 32 * b
                        seng = nc.vector if b < N_S_DVE else nc.gpsimd
                        seng.tensor_tensor(
                            out=S[:, off : off + WB + 2],
                            in0=X0[:, 0 : WB + 2],
                            in1=Z[:, d0 : d0 + WB + 2],
                            op=AluOpType.subtract,
                        )
                        off += WB + 2
                    A = wrk.tile([128, QW], f16)
                    nc.scalar.activation(
                        out=A, in_=S,
                        func=mybir.ActivationFunctionType.Abs,
                    )
                    col2 = g * NQ + q
                    col3 = NQ * NG + col2
                    if q < N_DVE_FORM:
                        # DVE-form: B + fused stt counts
                        Bt = wrk.tile([128, QW], f16)
                        nc.vector.tensor_scalar(
                            out=Bt, in0=A, scalar1=float(R), scalar2=None,
                            op0=AluOpType.is_le,
                        )
                        c2t = wrk.tile([128, QW], f16)
                        nc.vector.scalar_tensor_tensor(
                            out=c2t[:, 0 : QW - 1],
                            in0=A[:, 0 : QW - 1],
                            scalar=float(R),
                            in1=Bt[:, 1:QW],
                            op0=AluOpType.is_le,
                            op1=AluOpType.mult,
                            accum_out=stats[:, col2 : col2 + 1],
                        )
                        jt = wrk.tile([128, QW], f16)
                        nc.vector.scalar_tensor_tensor(
                            out=jt[:, 0 : QW - 2],
                            in0=A[:, 2:QW],
                            scalar=float(R),
                            in1=c2t[:, 0 : QW - 2],
                            op0=AluOpType.is_le,
                            op1=AluOpType.mult,
                            accum_out=stats[:, col3 : col3 + 1],
                        )
                    else:
                        # ACT-form: max-chain on DVE, Sign counts on ACT
                        m2t = wrk.tile([128, QW], f16)
                        nc.vector.tensor_tensor(
                            out=m2t[:, 0 : QW - 1],
                            in0=A[:, 0 : QW - 1],
                            in1=A[:, 1:QW],
                            op=AluOpType.max,
                        )
                        m3t = wrk.tile([128, QW], f16)
                        nc.vector.tensor_tensor(
                            out=m3t[:, 0 : QW - 2],
                            in0=m2t[:, 0 : QW - 2],
                            in1=A[:, 2:QW],
                            op=AluOpType.max,
                        )
                        jt = wrk.tile([128, QW], f16)
                        nc.scalar.activation(
                            out=jt[:, 0 : QW - 1],
                            in_=m2t[:, 0 : QW - 1],
                            func=mybir.ActivationFunctionType.Sign,
                            bias=rbias[:, 0:1],
                            scale=-1.0,
                            accum_out=stats[:, col2 : col2 + 1],
                        )
                        nc.scalar.activation(
                            out=jt[:, 0 : QW - 2],
                            in_=m3t[:, 0 : QW - 2],
                            func=mybir.ActivationFunctionType.Sign,
                            bias=rbias[:, 0:1],
                            scale=-1.0,
                            accum_out=stats[:, col3 : col3 + 1],
                        )

            pt = psum.tile([4, 2 * NQ * NG], f32)
            nc.tensor.matmul(pt, selt, stats, start=True, stop=True)
            red = singles.tile([4, 2 * NQ * NG], f32)
            nc.scalar.copy(out=red, in_=pt)
            nc.sync.dma_start(out=out[:, :], in_=red)

    _split_excess_waits(nc)
    return nc


def _get_nc():
    if "nc" not in _CACHE:
        _CACHE["nc"] = _build()
    return _CACHE["nc"]


def _get_runner():
    """Cached jitted 8-core executor: xpad [128, PADLEN] f32 -> [NCORES, 4, 64]."""
    if "fn" in _CACHE:
        return _CACHE["fn"]
    import jax
    import numpy as _np
    from jax.sharding import Mesh, PartitionSpec
    from jax.experimental.shard_map import shard_map
    import concourse.mybir as mybir
    from concourse.bass2jax import (
        _bass_exec_p,
        install_neuronx_cc_hook,
        partition_id_tensor,
    )

    nc = _get_nc()
    install_neuronx_cc_hook()

    in_names, out_names, out_avals, zero_outs = [], [], [], []
    partition_name = nc.partition_id_tensor.name if nc.partition_id_tensor else None
    for alloc in nc.m.functions[0].allocations:
        if not isinstance(alloc, mybir.MemoryLocationSet):
            continue
        name = alloc.memorylocations[0].name
        if alloc.kind == "ExternalInput":
            if name != partition_name:
                in_names.append(name)
        elif alloc.kind == "ExternalOutput":
            shape = tuple(alloc.tensor_shape)
            dtype = mybir.dt.np(alloc.dtype)
            out_names.append(name)
            out_avals.append(jax.core.ShapedArray(shape, dtype))
            zero_outs.append(_np.zeros(shape, dtype))
    n_params = len(in_names)
    n_outs = len(out_avals)
    all_in_names = list(in_names) + list(out_names) + (
        [partition_name] if partition_name else []
    )

    def _body(*args):
        operands = list(args)
        if partition_name is not None:
            operands.append(partition_id_tensor())
        return tuple(
            _bass_exec_p.bind(
                *operands,
                out_avals=tuple(out_avals),
                in_names=tuple(all_in_names),
                out_names=tuple(out_names),
                lowering_input_output_aliases=(),
                sim_require_finite=True,
                sim_require_nnan=True,
                nc=nc,
            )
        )

    devices = jax.devices("axon")[:NCORES]
    mesh = Mesh(np.asarray(devices), ("core",))
    in_specs = (PartitionSpec("core"),) * (n_params + n_outs)
    out_specs = (PartitionSpec("core"),) * n_outs
    fn = jax.jit(
        shard_map(
            _body, mesh=mesh, in_specs=in_specs, out_specs=out_specs, check_rep=False
        ),
        keep_unused=True,
    )
    concat_zeros = [
        np.zeros((NCORES * z.shape[0], *z.shape[1:]), z.dtype) for z in zero_outs
    ]

    def run(xpad):
        out = fn(xpad, *concat_zeros)
        arr = np.asarray(out[0])  # [NCORES*4, 64]
        return arr.reshape(NCORES, 4, 2 * NQ * NG)

    _CACHE["fn"] = run
    return run


def kernel(predictions, targets, _trace=False):
    global LAST_RESULTS

    preds = np.asarray(predictions, dtype=np.float32).reshape(B * C, T)
    targs = np.asarray(targets, dtype=np.float32).reshape(B * C, T)
    xall = np.concatenate([preds, targs], axis=0)  # [128, T]

    mu = xall.mean(axis=1, dtype=np.float64)
    sd = xall.std(axis=1, ddof=1, dtype=np.float64)
    xhat = ((xall - mu[:, None]) / (sd[:, None] + EPS)).astype(np.float32)

    xpad = np.empty((NSIG, PADLEN), dtype=np.float32)
    xpad[:, :T] = xhat
    xpad[:, T:] = 50000.0 + 100.0 * np.arange(PADLEN - T, dtype=np.float32)

    run = _get_runner()
    res = run(np.ascontiguousarray(xpad))
    LAST_RESULTS = res

    # Host spurious-c2 correction (exact fp16 mirror of device math)
    S1 = (xhat[:, 0:N] - xhat[:, N : N + 1]).astype(np.float16)
    S2 = (xhat[:, 1 : N + 1] - xhat[:, N + 1 : N + 2]).astype(np.float16)
    spur2 = (
        (np.abs(S1).astype(np.float32) <= np.float32(R))
        & (np.abs(S2).astype(np.float32) <= np.float32(R))
    ).sum(axis=1)  # [128]

    ents = np.zeros(NSIG, dtype=np.float64)
    for c in range(NCORES):
        o = res[c].astype(np.float64)  # [4, 64]
        for sl in range(S_PER_CORE):
            g, sig = sl // 4, sl % 4
            cnt2 = 0.0
            cnt3 = 0.0
            for q in range(NQ):
                a2 = o[sig, g * NQ + q]
                a3 = o[sig, NQ * NG + g * NQ + q]
                if q < N_DVE_FORM:
                    cnt2 += a2
                    cnt3 += a3
                else:
                    cnt2 += (a2 + 32.0 * (QW - 1)) / 2.0
                    cnt3 += (a3 + 32.0 * (QW - 2)) / 2.0
            sg = S_PER_CORE * c + sl
            m = 2.0 * (cnt2 - spur2[sg]) + N
            m1 = 2.0 * cnt3 + N
            ratio = m1 / max(m, 1.0)
            ent = -np.log(max(ratio, 1e-30)) if (m > 0 and m1 > 0) else 0.0
            ents[sg] = ent

    ep = ents[: B * C].reshape(B, C)
    et = ents[B * C :].reshape(B, C)
    return np.array(np.mean((ep - et) ** 2), dtype=np.float32)
